# revision 35
# baseline (speedup 1.0000x reference)
"""OHEM cross-entropy loss kernel for Trainium2 (8 NeuronCores, Bass/Tile).

Math (matches reference.py):
    logp   = log_softmax(seg_logit, axis=1)          # [B,C,H,W], C=19
    x_l    = logp at label (ignore 255 -> class 0)
    prob   = exp(x_l)
    thr    = max(sort(prob.flatten())[MIN_KEPT*B], 0.7)
    loss   = mean(-x_l * (prob < thr))

Device strategy (data-parallel over B across 8 cores, one image per core):
    Per pixel: t = x_raw[label] - log(sum_c exp(x_raw[c])), u = t - log(0.7),
    w = 1[u < 0].  Device accumulates per-partition partials of
    sum(relu(-u)) (= -sum(min(u,0)) = -sum(u*w)) and sum(sign(u))
    (-> count of u<0); the host combines:
        loss = (sum_relu - log(0.7)*count) / N

    (valid when count > MIN_KEPT*B, which the host verifies; otherwise an
    exact host fallback computes the quantile path)

Per 128xF-pixel chunk on-chip:
      - one DMA loads [128, 19, F] f32 logits (class-major, 4B*F rows)
      - ACT: ONE Exp instruction over all 19*F elems -> bf16 eb
      - DVE: pairwise bulk adds (2x bf16) -> sumexp;  ACT: lse = Ln(sumexp)
      - DVE: label gather as a 5-instruction mux tree of strided
        copy_predicated merges keyed on broadcast label bit-planes
      - DVE: u = (x_l - log.7) - lse
      - ACT: Relu(-u) and Sign(u), each with accum_out -> [P, 1] partials

All four activation funcs (Exp, Ln, Relu, Sign) live in the single
'natural_log_exp_and_others' table set; get_activation_tables is patched
during finalize so the act-table pass picks that set once instead of
thrashing exp_and_others <-> natural_log every chunk.
"""

import numpy as np

B = 8
C = 19
H, W = 512, 1024
HW = H * W            # 524288 pixels per image/core
P = 128               # SBUF partitions
FREE = HW // P        # 4096 pixels per partition
# Variable chunk schedule: small chunks at the start (pipeline fill: DVE
# starts sooner) and end (tail: last chunk's post-DMA chain is short);
# big 512-pixel chunks in the middle keep DMA rows at 2KB (345 GB/s).
CHUNKS = [128, 256, 384, 512, 512, 512, 512, 512, 384, 256, 128]
assert sum(CHUNKS) == FREE
NCHUNK = len(CHUNKS)  # 11
NBITS = 5             # ceil(log2(19))
C0 = float(np.log(np.float32(0.7)))
MIN_KEPT = 100000
IGNORE_INDEX = 255
N_TOTAL = B * HW

_CACHE = {}


def _build_nc():
    import concourse.bacc as bacc
    import concourse.mybir as mybir
    import concourse.tile as tile

    fp32 = mybir.dt.float32
    bf16 = mybir.dt.bfloat16
    u8 = mybir.dt.uint8
    AF = mybir.ActivationFunctionType

    nc = bacc.Bacc()
    logit = nc.dram_tensor("logit", [C, HW], fp32, kind="ExternalInput")
    bits = nc.dram_tensor("bits", [NBITS, P, FREE], u8, kind="ExternalInput")
    acc = nc.dram_tensor("acc", [P, 2 * NCHUNK], fp32, kind="ExternalOutput")

    # [C, (P FREE)] -> [P, C, FREE] view for chunked class-major loads
    logit_v = logit[:, :].rearrange("c (p f) -> p c f", p=P)

    with tile.TileContext(nc) as tc:
        with (
            tc.tile_pool(name="lb", bufs=3) as lb_pool,
            tc.tile_pool(name="eb", bufs=2) as eb_pool,
            tc.tile_pool(name="bits", bufs=3) as bits_pool,
            tc.tile_pool(name="pix", bufs=2) as pix_pool,
            tc.tile_pool(name="accp", bufs=1) as acc_pool,
        ):
            acc_t = acc_pool.tile([P, 2 * NCHUNK], fp32)

            # deferred per-chunk tails so ACT's in-order queue never blocks
            # behind DVE: relu/sign of chunk j are traced after exp of j+1
            pend = []

            def flush_tail():
                for u_, j_, fsz in pend:
                    scr = pix_pool.tile([P, fsz], fp32, tag="scr")
                    # sum(relu(-u)) = -sum(min(u, 0)) = -sum(u * 1[u<0])
                    nc.scalar.activation(
                        out=scr[:], in_=u_[:], func=AF.Relu, scale=-1.0,
                        accum_out=acc_t[:, j_ : j_ + 1],
                    )
                    scr2 = pix_pool.tile([P, fsz], fp32, tag="scr2")
                    # sum(sign(u)) -> count(u<0) = (N - total)/2 on host
                    nc.scalar.activation(
                        out=scr2[:], in_=u_[:], func=AF.Sign,
                        accum_out=acc_t[:, NCHUNK + j_ : NCHUNK + j_ + 1],
                    )
                pend.clear()

            off = 0
            for j, F in enumerate(CHUNKS):
                lb = lb_pool.tile([P, C, F], fp32, tag="lb")
                nc.sync.dma_start(out=lb[:], in_=logit_v[:, :, off : off + F])

                # bit-plane load on the ACT HWDGE ring: keeps the Sync ring
                # free for the big logit chunks
                bits_c = bits_pool.tile([P, NBITS, F], u8, tag="bits")
                nc.scalar.dma_start(
                    out=bits_c[:],
                    in_=bits[:, :, off : off + F].rearrange("k p f -> p k f"),
                )

                # one Exp over the whole [P, 19*F] chunk, f32 -> bf16.
                # Ramp chunks split it by class halves so the mux's lower
                # merges (WAR only vs classes 0-9) can start ~4us earlier.
                eb = eb_pool.tile([P, C, F], bf16, tag="eb")
                if j < 5:
                    nc.scalar.activation(
                        out=eb[:, 0:6, :], in_=lb[:, 0:6, :], func=AF.Exp
                    )
                    nc.scalar.activation(
                        out=eb[:, 6:12, :], in_=lb[:, 6:12, :], func=AF.Exp
                    )
                    nc.scalar.activation(
                        out=eb[:, 12:19, :], in_=lb[:, 12:19, :], func=AF.Exp
                    )
                else:
                    nc.scalar.activation(out=eb[:], in_=lb[:], func=AF.Exp)

                # drain previous chunk's reductions now that exp(j) is queued
                flush_tail()

                # label mux-tree gather, in place on lb (after exp read it);
                # each level is ONE strided copy_predicated with the bit-plane
                # mask broadcast across the merged slot pairs
                bs = bits_c[:]  # [P, NBITS, F]

                def mask(k, n):
                    return bs[:, k, :].unsqueeze(1).broadcast_to([P, n, F])

                # L0 (bit 0): slots {0,2,..,16} <- {1,3,..,17}; on ramp
                # chunks split at class 10 to match the split Exp's WAR
                if j < 5:
                    nc.vector.copy_predicated(
                        out=lb[:, 0:5:2, :], mask=mask(0, 3),
                        data=lb[:, 1:6:2, :],
                    )
                    nc.vector.copy_predicated(
                        out=lb[:, 6:11:2, :], mask=mask(0, 3),
                        data=lb[:, 7:12:2, :],
                    )
                    nc.vector.copy_predicated(
                        out=lb[:, 12:17:2, :], mask=mask(0, 3),
                        data=lb[:, 13:18:2, :],
                    )
                else:
                    nc.vector.copy_predicated(
                        out=lb[:, 0:18:2, :], mask=mask(0, 9),
                        data=lb[:, 1:19:2, :],
                    )
                # L1 (bit 1): {0,4,8,12,16} <- {2,6,10,14,18}
                nc.vector.copy_predicated(
                    out=lb[:, 0:17:4, :], mask=mask(1, 5), data=lb[:, 2:19:4, :]
                )
                # L2 (bit 2): {0,8} <- {4,12}
                nc.vector.copy_predicated(
                    out=lb[:, 0:9:8, :], mask=mask(2, 2), data=lb[:, 4:13:8, :]
                )
                # L3 (bit 3): {0} <- {8}
                nc.vector.copy_predicated(
                    out=lb[:, 0, :], mask=bs[:, 3, :], data=lb[:, 8, :]
                )
                # L4 (bit 4): {0} <- {16}
                nc.vector.copy_predicated(
                    out=lb[:, 0, :], mask=bs[:, 4, :], data=lb[:, 16, :]
                )

                # sumexp tree, 5 ops / 18F elems (bf16 2x adds):
                # [0:9]+=[10:19]; [0:5]+=[5:10]; [0:2]+=[3:5]; [0]+=[2];
                # sumexp = [0]+[1]
                nc.vector.tensor_tensor(
                    out=eb[:, 0:9, :], in0=eb[:, 0:9, :], in1=eb[:, 10:19, :],
                    op=mybir.AluOpType.add,
                )
                nc.vector.tensor_tensor(
                    out=eb[:, 0:5, :], in0=eb[:, 0:5, :], in1=eb[:, 5:10, :],
                    op=mybir.AluOpType.add,
                )
                nc.vector.tensor_tensor(
                    out=eb[:, 0:2, :], in0=eb[:, 0:2, :], in1=eb[:, 3:5, :],
                    op=mybir.AluOpType.add,
                )
                nc.vector.tensor_tensor(
                    out=eb[:, 0, :], in0=eb[:, 0, :], in1=eb[:, 2, :],
                    op=mybir.AluOpType.add,
                )
                sumexp = pix_pool.tile([P, F], bf16, tag="sumexp")
                nc.vector.tensor_tensor(
                    out=sumexp[:], in0=eb[:, 0, :], in1=eb[:, 1, :],
                    op=mybir.AluOpType.add,
                )

                lse = pix_pool.tile([P, F], fp32, tag="lse")
                nc.scalar.activation(out=lse[:], in_=sumexp[:], func=AF.Ln)

                # u = (x_l - log0.7) - lse
                u = pix_pool.tile([P, F], fp32, tag="u")
                nc.vector.scalar_tensor_tensor(
                    out=u[:], in0=lb[:, 0, :], scalar=C0, in1=lse[:],
                    op0=mybir.AluOpType.subtract, op1=mybir.AluOpType.subtract,
                )
                pend.append((u, j, F))
                off += F

            flush_tail()
            nc.sync.dma_start(out=acc[:, :], in_=acc_t[:])

    # Patch the act-table map so the insert_act_table_loads fixpoint picks
    # the one set containing ALL our funcs (Exp, Ln, Relu, Sign) instead of
    # thrashing exp_and_others <-> natural_log on every chunk. Indices of
    # the sets (= act_func_set_id) are preserved; only membership of the
    # non-target sets is masked.
    import concourse.bacc as bacc_mod
    import concourse.hw_specs as hw_mod

    AF = mybir.ActivationFunctionType
    target = "natural_log_exp_and_others"
    need = {AF.Exp, AF.Ln, AF.Relu, AF.Sign}
    orig = hw_mod.get_activation_tables

    def patched(arch):
        tabs = orig(arch)
        if target not in tabs or not need.issubset(tabs[target]):
            return tabs  # unexpected act_info; fall back to default behavior
        return {
            k: (v if k == target else {f for f in v if f not in need})
            for k, v in tabs.items()
        }

    bacc_mod.get_activation_tables = patched
    hw_mod.get_activation_tables = patched
    try:
        nc.finalize()  # Bacc: runs compile() (reg alloc, act-table pass, ...)
    finally:
        bacc_mod.get_activation_tables = orig
        hw_mod.get_activation_tables = orig
    return nc


def _host_fallback(seg_logit, seg_label):
    """Exact numpy replication of the reference (quantile path included)."""
    x = np.asarray(seg_logit, dtype=np.float32)
    lbl = np.asarray(seg_label)
    Bn, Cn = x.shape[0], x.shape[1]
    xf = x.reshape(Bn, Cn, -1)
    m = xf.max(axis=1, keepdims=True)
    e = np.exp(xf - m)
    lse = np.log(e.sum(axis=1, keepdims=True)) + m
    logp = xf - lse
    l2 = np.where(lbl == IGNORE_INDEX, 0, lbl).reshape(Bn, 1, -1).astype(np.int64)
    lp_at = np.take_along_axis(logp, l2, axis=1)[:, 0]
    prob = np.exp(lp_at)
    sortp = np.sort(prob.reshape(-1))
    idx = min(MIN_KEPT * Bn, sortp.shape[0] - 1)
    thr = max(float(sortp[idx]), np.float32(0.7))
    wgt = (prob < thr).astype(np.float32)
    return np.float32((-lp_at * wgt).mean())


def kernel(seg_logit, seg_label):
    from concourse import bass_utils

    x = np.ascontiguousarray(np.asarray(seg_logit, dtype=np.float32)).reshape(
        B, C, HW
    )
    lbl = np.asarray(seg_label)
    lbl = np.where(lbl == IGNORE_INDEX, 0, lbl).astype(np.uint8).reshape(B, P, FREE)
    # 5 bit-planes per core: [NBITS, P, FREE] uint8
    bits = np.stack(
        [((lbl >> k) & 1).astype(np.uint8) for k in range(NBITS)], axis=1
    )  # [B, NBITS, P, FREE]

    if "nc" not in _CACHE:
        _CACHE["nc"] = _build_nc()
    nc = _CACHE["nc"]

    in_maps = [{"logit": x[b], "bits": bits[b]} for b in range(B)]
    res = bass_utils.run_bass_kernel_spmd(nc, in_maps, core_ids=list(range(B)))

    relu_sum = 0.0
    sign_sum = 0.0
    for r in res.results:
        a = r["acc"]
        relu_sum += float(a[:, :NCHUNK].sum(dtype=np.float64))
        sign_sum += float(a[:, NCHUNK:].sum(dtype=np.float64))

    # count(u<0) from sum(sign(u)) (u==0 is measure-zero for this input)
    wacc = (N_TOTAL - sign_sum) / 2.0

    if wacc <= MIN_KEPT * B:
        # quantile threshold exceeds 0.7 -> exact host path (rare/never for
        # the target distribution)
        return _host_fallback(seg_logit, seg_label)

    # sum(-t*w) = sum(relu(-u)) - log(0.7)*count
    total = relu_sum - C0 * wacc
    return np.float32(total / N_TOTAL)
